# revision 1
# baseline (speedup 1.0000x reference)
"""Trainium2 Bass kernel for nn_MultiHeadAttention_63015760167496.

Computation (see reference): qkv = x @ Wqkv; RoPE on q,k; causal softmax
attention per head; out = einsum('bhts,bshd->bhtd', probs, v);
out.reshape(B,T,C) @ Wout  -- NOTE the reshape is a *head-major* flatten of
[B,H,T,D] into [B,T,C], so final-output row r = h*128 + t//16 depends only on
head h.  Sharding: head-parallel over 8 cores (2 heads/core); every core
computes its two heads end-to-end and produces final-output rows
[256*i, 256*i+256).  Host concatenates -- no collectives.

All big matmuls run as float32r (TF32-like) on the tensor engine.
Attention is computed in S^T layout ([s,t]): softmax denominator via a
ones-vector matmul (partition reduction on the PE), normalization via a K=1
broadcast matmul.  No running max is needed (scores are O(5), fp32 psum).
Host pre-arranges x^T and the weight slices so every DMA is 128 partitions
x >=16KB contiguous.
"""

import math
import sys

for _p in ("/opt/trn_rl_repo", "/root/.axon_site/_ro/trn_rl_repo"):
    if _p not in sys.path:
        sys.path.insert(0, _p)

import numpy as np

import concourse.bass as bass
import concourse.mybir as mybir
import concourse.tile as tile
from concourse import bacc
from concourse.bass_utils import run_bass_kernel_spmd

B, T, C = 2, 2048, 2048
H = 16            # heads total
D = C // H        # 128 head dim
HALF = D // 2     # 64
P = 128
KO = C // P       # 16 contraction chunks
NCORES = 8
HPC = H // NCORES  # 2 heads per core
TQ = 256          # t-tile for qkv projection
NT = T // TQ
TA = 256          # t-tile for attention
NSC = T // P      # 16 s-chunks
ROPE_BASE = 10000.0
SCALE = 1.0 / math.sqrt(D)

f32 = mybir.dt.float32
f32r = mybir.dt.float32r


def _build():
    nc = bacc.Bacc("TRN2", target_bir_lowering=False, debug=False,
                   num_devices=NCORES)

    # host-pre-tiled x^T: xTt[b, ti, p, ko, u] = x[b, ti*TQ+u, ko*128+p]
    xTt = nc.dram_tensor("xTt", [B, NT, P, KO, TQ], f32r, kind="ExternalInput")
    # host-pre-chunked weights: w[p, ko, m] = W[ko*128+p, m]
    wq = nc.dram_tensor("wq", [P, KO, HPC * D], f32r, kind="ExternalInput")
    wk = nc.dram_tensor("wk", [P, KO, HPC * D], f32r, kind="ExternalInput")
    wv = nc.dram_tensor("wv", [P, KO, HPC * D], f32r, kind="ExternalInput")
    wout = nc.dram_tensor("wout", [C, C], f32r, kind="ExternalInput")
    cs2 = nc.dram_tensor("cs2", [P, T], f32, kind="ExternalInput")  # [cos;cos]
    sn1 = nc.dram_tensor("sn1", [HALF, T], f32, kind="ExternalInput")  # sin
    maskM = nc.dram_tensor("maskM", [P, P], f32r, kind="ExternalInput")
    y = nc.dram_tensor("y", [B, HPC * D, C], f32, kind="ExternalOutput")

    with tile.TileContext(nc) as tc:
        with tc.tile_pool(name="const", bufs=1) as cp_, \
             tc.tile_pool(name="qkv", bufs=1) as qp, \
             tc.tile_pool(name="ot", bufs=1) as op_, \
             tc.tile_pool(name="small", bufs=2) as sp:

            wq_sb = cp_.tile([P, KO, HPC * D], f32r, tag="wq")
            wk_sb = cp_.tile([P, KO, HPC * D], f32r, tag="wk")
            wv_sb = cp_.tile([P, KO, HPC * D], f32r, tag="wv")
            nc.sync.dma_start(wq_sb[:], wq.ap())
            cs_sb = cp_.tile([P, T], f32, tag="cs")
            sn_sb = cp_.tile([HALF, T], f32, tag="sn")
            mask_sb = cp_.tile([P, P], f32r, tag="mask")
            ones_row = cp_.tile([1, P], f32, tag="ones_row")
            nc.vector.memset(ones_row[:], 1.0)
            ones_rowr = cp_.tile([1, P], f32r, tag="ones_rowr")
            nc.vector.tensor_copy(ones_rowr[:], ones_row[:])
            ones_f32 = cp_.tile([P, 1], f32, tag="ones_f32")
            nc.vector.memset(ones_f32[:], 1.0)
            ones_col = cp_.tile([P, 1], f32r, tag="ones_col")
            nc.vector.tensor_copy(ones_col[:], ones_f32[:])

            # persistent attention outputs O^T per (b, local head): [d, t]
            oT = [[op_.tile([P, T], f32r, tag=f"oT{b}{hh}", name=f"oT{b}{hh}")
                   for hh in range(HPC)] for b in range(B)]

            for b in range(B):
                qT = [qp.tile([P, T], f32r, tag=f"qT{hh}", name=f"qT{b}{hh}")
                      for hh in range(HPC)]
                kT = [qp.tile([P, T], f32r, tag=f"kT{hh}", name=f"kT{b}{hh}")
                      for hh in range(HPC)]
                vt = [qp.tile([P, NSC, D], f32r, tag=f"v{hh}", name=f"v{b}{hh}")
                      for hh in range(HPC)]

                # ---------------- QKV projection + RoPE ----------------
                with tc.tile_pool(name=f"xt{b}", bufs=2) as xp, \
                     tc.tile_pool(name=f"psA{b}", bufs=4, space="PSUM") as psa, \
                     tc.tile_pool(name=f"rope{b}", bufs=3) as rp:
                    for ti in range(NT):
                        sl = slice(ti * TQ, (ti + 1) * TQ)
                        xt = xp.tile([P, KO, TQ], f32r, tag="xt")
                        nc.sync.dma_start(xt[:], xTt.ap()[b, ti])
                        cs = cs_sb[:, sl]
                        sn = sn_sb[:, sl]  # [64, TQ] base partition 0

                        def qkmm(w_sb, hh):
                            hsl = slice(hh * D, (hh + 1) * D)
                            ps = psa.tile([P, TQ], f32, tag="acc",
                                          name=f"acc{b}_{ti}_{hh}")
                            for ko in range(KO):
                                nc.tensor.matmul(ps[:], w_sb[:, ko, hsl],
                                                 xt[:, ko, :],
                                                 start=(ko == 0),
                                                 stop=(ko == KO - 1))
                            return ps

                        def rope(ps, dst):
                            # tcos = ps * [cos;cos] (one full mult); tsw
                            # pre-swaps halves: tsw[0:64]=q2*sin,
                            # tsw[64:128]=q1*sin so the gpsimd add/sub reads
                            # align on base partitions.
                            tcos = rp.tile([P, TQ], f32, tag="tcos")
                            tsw = rp.tile([P, TQ], f32, tag="tsw")
                            nc.vector.tensor_mul(tcos[:], ps[:], cs)
                            nc.vector.tensor_mul(tsw[0:HALF, :],
                                                 ps[HALF:P, :], sn)
                            nc.vector.tensor_mul(tsw[HALF:P, :],
                                                 ps[0:HALF, :], sn)
                            nc.gpsimd.tensor_sub(dst[0:HALF, sl],
                                                 tcos[0:HALF, :],
                                                 tsw[0:HALF, :])
                            nc.gpsimd.tensor_add(dst[HALF:P, sl],
                                                 tcos[HALF:P, :],
                                                 tsw[HALF:P, :])

                        if b == 0 and ti == 0:
                            # q accums first (need only wq + xt0); stagger the
                            # remaining const DMAs behind them so the first
                            # matmuls aren't starved for DMA bandwidth.
                            psq = [qkmm(wq_sb, hh) for hh in range(HPC)]
                            nc.sync.dma_start(cs_sb[:], cs2.ap())
                            nc.sync.dma_start(sn_sb[:], sn1.ap())
                            nc.sync.dma_start(wk_sb[:], wk.ap())
                            nc.sync.dma_start(wv_sb[:], wv.ap())
                            nc.sync.dma_start(mask_sb[:], maskM.ap())
                            for hh in range(HPC):
                                rope(psq[hh], qT[hh])
                                psk = qkmm(wk_sb, hh)
                                rope(psk, kT[hh])
                        else:
                            for hh in range(HPC):
                                for w_sb, dst in ((wq_sb, qT[hh]),
                                                  (wk_sb, kT[hh])):
                                    rope(qkmm(w_sb, hh), dst)
                        for sub in range(TQ // P):
                            psv = psa.tile([P, HPC * D], f32, tag="acc")
                            for ko in range(KO):
                                nc.tensor.matmul(
                                    psv[:], xt[:, ko, sub * P:(sub + 1) * P],
                                    wv_sb[:, ko, :],
                                    start=(ko == 0), stop=(ko == KO - 1))
                            tci = ti * (TQ // P) + sub
                            for hh in range(HPC):
                                nc.vector.tensor_copy(
                                    vt[hh][:, tci, :],
                                    psv[:, hh * D:(hh + 1) * D])

                # ---------------- attention (S^T layout) ----------------
                with tc.tile_pool(name=f"psBsc{b}", bufs=3, space="PSUM") as pssc, \
                     tc.tile_pool(name=f"psBo{b}", bufs=2, space="PSUM") as pso, \
                     tc.tile_pool(name=f"psBsum{b}", bufs=2, space="PSUM") as pssum, \
                     tc.tile_pool(name=f"psBbc{b}", bufs=1, space="PSUM") as psbc, \
                     tc.tile_pool(name=f"pt{b}", bufs=3) as ptp:
                    for hh in range(HPC):
                        for ta in range(T // TA):
                            tsl = slice(ta * TA, (ta + 1) * TA)
                            ps_o = pso.tile([P, TA], f32, tag="o")
                            ps_sum = pssum.tile([1, TA], f32, tag="sum")
                            smax = (ta + 1) * (TA // P) - 1
                            for s in range(smax + 1):
                                diag = s >= ta * (TA // P)
                                t_lo = (s - ta * (TA // P)) * P if diag else 0
                                w = slice(t_lo, TA)
                                qsl = slice(ta * TA + t_lo, (ta + 1) * TA)
                                ps_sc = pssc.tile([P, TA], f32, tag="sc")
                                nc.tensor.matmul(
                                    ps_sc[:, w], kT[hh][:, s * P:(s + 1) * P],
                                    qT[hh][:, qsl], start=True, stop=True)
                                pt = ptp.tile([P, TA], f32r, tag="pt")
                                nc.scalar.activation(
                                    pt[:, w], ps_sc[:, w],
                                    mybir.ActivationFunctionType.Exp,
                                    scale=SCALE)
                                if diag:  # mask the 128x128 triangle
                                    nc.vector.tensor_mul(
                                        pt[:, t_lo:t_lo + P],
                                        pt[:, t_lo:t_lo + P], mask_sb[:])
                                first, last = (s == 0), (s == smax)
                                nc.tensor.matmul(ps_o[:, w], vt[hh][:, s, :],
                                                 pt[:, w],
                                                 start=first, stop=last)
                                nc.tensor.matmul(ps_sum[:, w], ones_col[:],
                                                 pt[:, w],
                                                 start=first, stop=last)
                            recf = sp.tile([1, TA], f32, tag="recf")
                            nc.vector.reciprocal_approx_fast(recf[:], ps_sum[:])
                            rec = sp.tile([1, TA], f32r, tag="rec")
                            nc.vector.tensor_copy(rec[:], recf[:])
                            ps_bc = psbc.tile([P, TA], f32, tag="bc")
                            nc.tensor.matmul(ps_bc[:], ones_rowr[:], rec[:],
                                             start=True, stop=True)
                            bc_sb = sp.tile([P, TA], f32, tag="bc_sb")
                            nc.scalar.copy(bc_sb[:], ps_bc[:])
                            # write oT pre-shuffled for the out-projection:
                            # oT[p, j*128+u] = O^T[p, t=u*16+j]
                            oview = oT[b][hh].rearrange(
                                "p (j u) -> p u j", j=KO)[
                                :, (TA // 16) * ta:(TA // 16) * (ta + 1), :]
                            nc.vector.tensor_mul(
                                oview,
                                ps_o[:].rearrange("p (u j) -> p u j", j=KO),
                                bc_sb[:].rearrange("p (u j) -> p u j", j=KO))

            # ---------------- output projection ----------------
            TC_ = 256
            with tc.tile_pool(name="woutp", bufs=1) as wop, \
                 tc.tile_pool(name="psC", bufs=4, space="PSUM") as psc:
                for cpi in range(C // TC_):
                    csl = slice(cpi * TC_, (cpi + 1) * TC_)
                    wts = []
                    for j in range(KO):
                        wt = wop.tile([P, TC_], f32r, tag=f"w{j}",
                                      name=f"w{cpi}_{j}")
                        nc.sync.dma_start(wt[:], wout.ap()[j * P:(j + 1) * P, csl])
                        wts.append(wt)
                    for b in range(B):
                        for hh in range(HPC):
                            psy = psc.tile([P, TC_], f32, tag="y")
                            for j in range(KO):
                                nc.tensor.matmul(psy[:],
                                                 oT[b][hh][:, j * P:(j + 1) * P],
                                                 wts[j][:],
                                                 start=(j == 0),
                                                 stop=(j == KO - 1))
                            ysb = sp.tile([P, TC_], f32, tag="ysb")
                            nc.vector.tensor_copy(ysb[:], psy[:])
                            nc.sync.dma_start(
                                y.ap()[b, hh * D:(hh + 1) * D, csl], ysb[:])

    nc.compile()
    return nc


_NC = None


def _get_nc():
    global _NC
    if _NC is None:
        _NC = _build()
    return _NC


def _host_tables():
    pos = np.arange(T, dtype=np.float32)[:, None]
    div = np.exp(np.arange(0, 2 * HALF, 2, dtype=np.float32)
                 * np.float32(-math.log(ROPE_BASE) / (2 * HALF)))
    ang = pos * div[None, :]
    cosv = np.cos(ang).astype(np.float32)   # [T, HALF]
    sinv = np.sin(ang).astype(np.float32)
    cosT = np.ascontiguousarray(cosv.T)     # [HALF, T]
    sinT = np.ascontiguousarray(sinv.T)
    cs2 = np.ascontiguousarray(np.concatenate([cosT, cosT], axis=0))  # [P, T]
    sn1 = sinT
    # triangle mask M[s, w] = 1 iff s <= w
    ww = np.arange(P)[None, :]
    ss = np.arange(P)[:, None]
    maskM = (ss <= ww).astype(np.float32)
    return cs2, sn1, maskM


def _make_in_maps(x, Wqkv, Wout):
    x = np.asarray(x, dtype=np.float32)
    Wqkv = np.asarray(Wqkv, dtype=np.float32)
    Wout = np.asarray(Wout, dtype=np.float32)
    assert x.shape == (B, T, C) and Wqkv.shape == (C, 3 * C) \
        and Wout.shape == (C, C)

    cs2, sn1, maskM = _host_tables()
    # xTt[b, ti, p, ko, u] = x[b, ti*TQ+u, ko*128+p]
    xTt = np.ascontiguousarray(
        x.reshape(B, NT, TQ, KO, P).transpose(0, 1, 4, 3, 2))

    in_maps = []
    for core in range(NCORES):
        h0 = core * HPC
        cols = slice(h0 * D, (h0 + HPC) * D)
        ws = []
        for part in range(3):
            w = Wqkv[:, part * C:(part + 1) * C][:, cols]  # [C, HPC*D]
            ws.append(np.ascontiguousarray(
                w.reshape(KO, P, HPC * D).transpose(1, 0, 2)))
        in_maps.append({
            "xTt": xTt,
            "wq": ws[0], "wk": ws[1], "wv": ws[2],
            "wout": Wout,
            "cs2": cs2, "sn1": sn1, "maskM": maskM,
        })
    return in_maps


def _run(x, Wqkv, Wout, trace=False):
    nc = _get_nc()
    in_maps = _make_in_maps(x, Wqkv, Wout)
    res = run_bass_kernel_spmd(nc, in_maps, core_ids=list(range(NCORES)),
                               trace=trace)
    out = np.empty((B, T, C), dtype=np.float32)
    for core in range(NCORES):
        out[:, core * HPC * D:(core + 1) * HPC * D, :] = \
            res.results[core]["y"]
    return out, res


def kernel(x, Wqkv, Wout):
    out, _ = _run(x, Wqkv, Wout)
    return out



# revision 2
# speedup vs baseline: 1.2572x; 1.2572x over previous
"""Trainium2 Bass kernel for nn_MultiHeadAttention_63015760167496.

Computation (see reference): qkv = x @ Wqkv; RoPE on q,k; causal softmax
attention per head; out = einsum('bhts,bshd->bhtd', probs, v);
out.reshape(B,T,C) @ Wout  -- the reshape is a *head-major* flatten of
[B,H,T,D] into [B,T,C], so final-output row r = h*128 + t//16 depends only
on head h.  Sharding: head-parallel over 8 cores (2 heads/core); every core
computes its two heads end-to-end and produces final-output rows
[256*i, 256*i+256).  Host concatenates -- no collectives.

All on-device data is bf16 (PSUM accumulation f32), which halves DMA/SBUF
vs f32r at the same 1 cycle/row PE rate.  Attention runs in S^T layout
([s,t]): softmax denominator via a ones-column matmul (partition reduction
on the PE), normalization broadcast via gpsimd.partition_broadcast (off the
PE/Act critical path).  The attention inner loop is software-pipelined: the
score matmuls for blocks s+1, s+2 are issued before the PV/sum matmuls of
block s, so the PE never stalls waiting for the Act-engine exp.  Wout is
prefetched into SBUF during attention so the output projection is pure PE.
"""

import math
import sys

for _p in ("/opt/trn_rl_repo", "/root/.axon_site/_ro/trn_rl_repo"):
    if _p not in sys.path:
        sys.path.insert(0, _p)

import numpy as np
import ml_dtypes

import concourse.bass as bass
import concourse.mybir as mybir
import concourse.tile as tile
from concourse import bacc
from concourse.bass_utils import run_bass_kernel_spmd

B, T, C = 2, 2048, 2048
H = 16            # heads total
D = C // H        # 128 head dim
HALF = D // 2     # 64
P = 128
KO = C // P       # 16 contraction chunks
NCORES = 8
HPC = H // NCORES  # 2 heads per core
TQ = 512          # t-tile for qkv projection
NT = T // TQ      # 4
TA = 512          # t-tile for attention
NTA = T // TA     # 4
NSC = T // P      # 16 s-chunks
ROPE_BASE = 10000.0
SCALE = 1.0 / math.sqrt(D)
TC_ = 512         # col-tile for out projection
LOOKAHEAD = 2     # attention software-pipeline depth

f32 = mybir.dt.float32
f32r = mybir.dt.float32r
bf16 = mybir.dt.bfloat16
EXP = mybir.ActivationFunctionType.Exp


def _build():
    nc = bacc.Bacc("TRN2", target_bir_lowering=False, debug=False,
                   num_devices=NCORES)

    # host-pre-tiled x^T: xTt[b, ti, p, ko, u] = x[b, ti*TQ+u, ko*128+p]
    xTt = nc.dram_tensor("xTt", [B, NT, P, KO, TQ], bf16, kind="ExternalInput")
    # host-pre-chunked weights: w[p, ko, m] = W[ko*128+p, m]
    wq = nc.dram_tensor("wq", [P, KO, HPC * D], bf16, kind="ExternalInput")
    wk = nc.dram_tensor("wk", [P, KO, HPC * D], bf16, kind="ExternalInput")
    wv = nc.dram_tensor("wv", [P, KO, HPC * D], bf16, kind="ExternalInput")
    # woutT[j, p, c] = Wout[j*128+p, c]
    woutT = nc.dram_tensor("woutT", [KO, P, C], bf16, kind="ExternalInput")
    cs2 = nc.dram_tensor("cs2", [P, T], f32, kind="ExternalInput")  # [cos;cos]
    sn1 = nc.dram_tensor("sn1", [HALF, T], f32, kind="ExternalInput")  # sin
    maskM = nc.dram_tensor("maskM", [P, P], bf16, kind="ExternalInput")
    y = nc.dram_tensor("y", [B, HPC * D, C], f32, kind="ExternalOutput")

    with tile.TileContext(nc) as tc:
        with tc.tile_pool(name="const", bufs=1) as cp_, \
             tc.tile_pool(name="wo", bufs=1) as wop, \
             tc.tile_pool(name="qkv", bufs=1) as qp, \
             tc.tile_pool(name="ot", bufs=1) as op_, \
             tc.tile_pool(name="small", bufs=2) as sp:

            wq_sb = cp_.tile([P, KO, HPC * D], bf16, tag="wq")
            wk_sb = cp_.tile([P, KO, HPC * D], bf16, tag="wk")
            wv_sb = cp_.tile([P, KO, HPC * D], bf16, tag="wv")
            cs_sb = cp_.tile([P, T], f32, tag="cs")
            sn_sb = cp_.tile([HALF, T], f32, tag="sn")
            mask_sb = cp_.tile([P, P], bf16, tag="mask")
            wout_sb = wop.tile([P, KO, C], bf16, tag="wout")

            # startup DMAs, chunked so the first matmul chain can start
            # after ~0.5MB instead of ~4MB: wq in 4 ko-groups.
            for g in range(4):
                nc.sync.dma_start(wq_sb[:, 4 * g:4 * g + 4, :],
                                  wq.ap()[:, 4 * g:4 * g + 4, :])

            ones_f32 = cp_.tile([P, 1], f32, tag="ones_f32")
            nc.vector.memset(ones_f32[:], 1.0)
            ones_col = cp_.tile([P, 1], bf16, tag="ones_col")
            nc.vector.tensor_copy(ones_col[:], ones_f32[:])
            # act-table warmup: force the Exp table load at t=0 instead of
            # in the middle of the first attention block.
            warm_in = cp_.tile([1, 8], f32, tag="warm_in")
            nc.vector.memset(warm_in[:], 0.0)
            warm_out = cp_.tile([1, 8], f32, tag="warm_out")
            nc.scalar.activation(warm_out[:], warm_in[:], EXP, scale=1.0)

            # persistent attention outputs O^T per (b, local head): [d, t]
            oT = [[op_.tile([P, T], bf16, tag=f"oT{b}{hh}", name=f"oT{b}{hh}")
                   for hh in range(HPC)] for b in range(B)]

            for b in range(B):
                qT = [qp.tile([P, T], bf16, tag=f"qT{hh}", name=f"qT{b}{hh}")
                      for hh in range(HPC)]
                kT = [qp.tile([P, T], bf16, tag=f"kT{hh}", name=f"kT{b}{hh}")
                      for hh in range(HPC)]
                vt = [qp.tile([P, NSC, D], bf16, tag=f"v{hh}", name=f"v{b}{hh}")
                      for hh in range(HPC)]

                # ---------------- QKV projection + RoPE ----------------
                with tc.tile_pool(name=f"xt{b}", bufs=2) as xp, \
                     tc.tile_pool(name=f"psA{b}", bufs=3, space="PSUM") as psa, \
                     tc.tile_pool(name=f"psV{b}", bufs=2, space="PSUM") as psv_p, \
                     tc.tile_pool(name=f"rope{b}", bufs=3) as rp:
                    for ti in range(NT):
                        sl = slice(ti * TQ, (ti + 1) * TQ)
                        xt = xp.tile([P, KO, TQ], bf16, tag="xt")
                        if b == 0 and ti == 0:
                            for g in range(4):
                                nc.sync.dma_start(
                                    xt[:, 4 * g:4 * g + 4, :],
                                    xTt.ap()[b, ti, :, 4 * g:4 * g + 4, :])
                        else:
                            nc.sync.dma_start(xt[:], xTt.ap()[b, ti])
                        cs = cs_sb[:, sl]
                        sn = sn_sb[:, sl]  # [64, TQ] base partition 0

                        def qkmm(w_sb, hh):
                            hsl = slice(hh * D, (hh + 1) * D)
                            ps = psa.tile([P, TQ], f32, tag="acc",
                                          name=f"acc{b}_{ti}_{hh}")
                            for ko in range(KO):
                                nc.tensor.matmul(ps[:], w_sb[:, ko, hsl],
                                                 xt[:, ko, :],
                                                 start=(ko == 0),
                                                 stop=(ko == KO - 1))
                            return ps

                        def rope(ps, dst):
                            # tcos = ps * [cos;cos]; tsw pre-swaps halves:
                            # tsw[0:64]=q2*sin, tsw[64:128]=q1*sin so the
                            # gpsimd add/sub reads align on base partitions.
                            tcos = rp.tile([P, TQ], bf16, tag="tcos")
                            tsw = rp.tile([P, TQ], bf16, tag="tsw")
                            nc.vector.tensor_mul(tcos[:], ps[:], cs)
                            nc.vector.tensor_mul(tsw[0:HALF, :],
                                                 ps[HALF:P, :], sn)
                            nc.vector.tensor_mul(tsw[HALF:P, :],
                                                 ps[0:HALF, :], sn)
                            nc.gpsimd.tensor_sub(dst[0:HALF, sl],
                                                 tcos[0:HALF, :],
                                                 tsw[0:HALF, :])
                            nc.gpsimd.tensor_add(dst[HALF:P, sl],
                                                 tcos[HALF:P, :],
                                                 tsw[HALF:P, :])

                        if b == 0 and ti == 0:
                            # q accums first (need only wq + xt0); stagger
                            # the remaining const DMAs behind them so the
                            # first matmuls aren't starved for bandwidth.
                            psq = [qkmm(wq_sb, hh) for hh in range(HPC)]
                            nc.sync.dma_start(cs_sb[:], cs2.ap())
                            nc.sync.dma_start(sn_sb[:], sn1.ap())
                            nc.sync.dma_start(wk_sb[:], wk.ap())
                            nc.sync.dma_start(wv_sb[:], wv.ap())
                            nc.sync.dma_start(mask_sb[:], maskM.ap())
                            for hh in range(HPC):
                                rope(psq[hh], qT[hh])
                                psk = qkmm(wk_sb, hh)
                                rope(psk, kT[hh])
                        else:
                            for hh in range(HPC):
                                for w_sb, dst in ((wq_sb, qT[hh]),
                                                  (wk_sb, kT[hh])):
                                    rope(qkmm(w_sb, hh), dst)
                        for sub in range(TQ // P):
                            psv = psv_p.tile([P, HPC * D], f32, tag="acc")
                            for ko in range(KO):
                                nc.tensor.matmul(
                                    psv[:], xt[:, ko, sub * P:(sub + 1) * P],
                                    wv_sb[:, ko, :],
                                    start=(ko == 0), stop=(ko == KO - 1))
                            tci = ti * (TQ // P) + sub
                            for hh in range(HPC):
                                # Act engine is idle during QKV; use it for
                                # the psum->sbuf v copies.
                                nc.scalar.copy(
                                    vt[hh][:, tci, :],
                                    psv[:, hh * D:(hh + 1) * D])

                # ---------------- attention (S^T layout) ----------------
                with tc.tile_pool(name=f"psBsc{b}", bufs=3, space="PSUM") as pssc, \
                     tc.tile_pool(name=f"psBo{b}", bufs=2, space="PSUM") as pso, \
                     tc.tile_pool(name=f"psBsum{b}", bufs=2, space="PSUM") as pssum, \
                     tc.tile_pool(name=f"pt{b}", bufs=LOOKAHEAD + 2) as ptp:
                    nwo = 0  # wout prefetch cursor (b == 0 only)
                    for hh in range(HPC):
                        for ta in range(NTA):
                            spt = TA // P  # s-chunks spanned by one t-tile
                            ps_o = pso.tile([P, TA], f32, tag="o")
                            ps_sum = pssum.tile([1, TA], f32, tag="sum")
                            smax = (ta + 1) * spt - 1
                            pend = []

                            def flush(last):
                                pt_, w_, s_, first_ = pend.pop(0)
                                nc.tensor.matmul(ps_o[:, w_],
                                                 vt[hh][:, s_, :],
                                                 pt_[:, w_],
                                                 start=first_, stop=last)
                                nc.tensor.matmul(ps_sum[:, w_], ones_col[:],
                                                 pt_[:, w_],
                                                 start=first_, stop=last)

                            for s in range(smax + 1):
                                diag = s >= ta * spt
                                t_lo = (s - ta * spt) * P if diag else 0
                                w = slice(t_lo, TA)
                                qsl = slice(ta * TA + t_lo, (ta + 1) * TA)
                                ps_sc = pssc.tile([P, TA], f32, tag="sc")
                                nc.tensor.matmul(
                                    ps_sc[:, w], kT[hh][:, s * P:(s + 1) * P],
                                    qT[hh][:, qsl], start=True, stop=True)
                                pt = ptp.tile([P, TA], bf16, tag="pt")
                                nc.scalar.activation(pt[:, w], ps_sc[:, w],
                                                     EXP, scale=SCALE)
                                if diag:  # mask the 128x128 triangle
                                    nc.vector.tensor_mul(
                                        pt[:, t_lo:t_lo + P],
                                        pt[:, t_lo:t_lo + P], mask_sb[:])
                                pend.append((pt, w, s, s == 0))
                                if len(pend) > LOOKAHEAD:
                                    flush(False)
                            while len(pend) > 1:
                                flush(False)
                            flush(True)

                            # normalization: recip on VE, partition
                            # broadcast on gpsimd, multiply on VE -- the PE
                            # is not involved.
                            recf = sp.tile([1, TA], f32, tag="recf")
                            nc.vector.reciprocal_approx_fast(recf[:],
                                                             ps_sum[:])
                            recb = sp.tile([1, TA], bf16, tag="recb")
                            nc.vector.tensor_copy(recb[:], recf[:])
                            bcb = sp.tile([P, TA], bf16, tag="bcb")
                            nc.gpsimd.partition_broadcast(bcb[:], recb[:],
                                                          channels=P)
                            # write oT pre-shuffled for the out-projection:
                            # oT[p, j*128+u] = O^T[p, t=u*16+j]
                            nu = TA // 16
                            oview = oT[b][hh].rearrange(
                                "p (j u) -> p u j", j=KO)[
                                :, nu * ta:nu * (ta + 1), :]
                            nc.vector.tensor_mul(
                                oview,
                                ps_o[:].rearrange("p (u j) -> p u j", j=KO),
                                bcb[:].rearrange("p (u j) -> p u j", j=KO))

                            # prefetch wout during attention of b=0: two
                            # j-blocks per (hh, ta) -> all 16 by the end.
                            if b == 0:
                                for _ in range(2):
                                    nc.sync.dma_start(
                                        wout_sb[:, nwo, :], woutT.ap()[nwo])
                                    nwo += 1

                # ---------------- output projection (pure PE) ----------
                with tc.tile_pool(name=f"psC{b}", bufs=3, space="PSUM") as psc:
                    for hh in range(HPC):
                        for cpi in range(C // TC_):
                            csl = slice(cpi * TC_, (cpi + 1) * TC_)
                            psy = psc.tile([P, TC_], f32, tag="y")
                            for j in range(KO):
                                nc.tensor.matmul(psy[:],
                                                 oT[b][hh][:, j * P:(j + 1) * P],
                                                 wout_sb[:, j, csl],
                                                 start=(j == 0),
                                                 stop=(j == KO - 1))
                            ysb = sp.tile([P, TC_], f32, tag="ysb")
                            nc.scalar.copy(ysb[:], psy[:])
                            nc.sync.dma_start(
                                y.ap()[b, hh * D:(hh + 1) * D, csl], ysb[:])

    nc.compile()
    return nc


_NC = None


def _get_nc():
    global _NC
    if _NC is None:
        _NC = _build()
    return _NC


def _host_tables():
    pos = np.arange(T, dtype=np.float32)[:, None]
    div = np.exp(np.arange(0, 2 * HALF, 2, dtype=np.float32)
                 * np.float32(-math.log(ROPE_BASE) / (2 * HALF)))
    ang = pos * div[None, :]
    cosv = np.cos(ang).astype(np.float32)   # [T, HALF]
    sinv = np.sin(ang).astype(np.float32)
    cosT = np.ascontiguousarray(cosv.T)     # [HALF, T]
    sinT = np.ascontiguousarray(sinv.T)
    cs2 = np.ascontiguousarray(np.concatenate([cosT, cosT], axis=0))  # [P, T]
    sn1 = sinT
    # triangle mask M[s, w] = 1 iff s <= w
    ww = np.arange(P)[None, :]
    ss = np.arange(P)[:, None]
    maskM = (ss <= ww).astype(ml_dtypes.bfloat16)
    return cs2, sn1, maskM


def _make_in_maps(x, Wqkv, Wout):
    x = np.asarray(x, dtype=np.float32)
    Wqkv = np.asarray(Wqkv, dtype=np.float32)
    Wout = np.asarray(Wout, dtype=np.float32)
    assert x.shape == (B, T, C) and Wqkv.shape == (C, 3 * C) \
        and Wout.shape == (C, C)

    cs2, sn1, maskM = _host_tables()
    # xTt[b, ti, p, ko, u] = x[b, ti*TQ+u, ko*128+p]
    xTt = np.ascontiguousarray(
        x.reshape(B, NT, TQ, KO, P).transpose(0, 1, 4, 3, 2)
    ).astype(ml_dtypes.bfloat16)
    woutT = np.ascontiguousarray(
        Wout.reshape(KO, P, C)).astype(ml_dtypes.bfloat16)

    in_maps = []
    for core in range(NCORES):
        h0 = core * HPC
        cols = slice(h0 * D, (h0 + HPC) * D)
        ws = []
        for part in range(3):
            w = Wqkv[:, part * C:(part + 1) * C][:, cols]  # [C, HPC*D]
            ws.append(np.ascontiguousarray(
                w.reshape(KO, P, HPC * D).transpose(1, 0, 2)
            ).astype(ml_dtypes.bfloat16))
        in_maps.append({
            "xTt": xTt,
            "wq": ws[0], "wk": ws[1], "wv": ws[2],
            "woutT": woutT,
            "cs2": cs2, "sn1": sn1, "maskM": maskM,
        })
    return in_maps


def _run(x, Wqkv, Wout, trace=False):
    nc = _get_nc()
    in_maps = _make_in_maps(x, Wqkv, Wout)
    res = run_bass_kernel_spmd(nc, in_maps, core_ids=list(range(NCORES)),
                               trace=trace)
    out = np.empty((B, T, C), dtype=np.float32)
    for core in range(NCORES):
        out[:, core * HPC * D:(core + 1) * HPC * D, :] = \
            res.results[core]["y"]
    return out, res


def kernel(x, Wqkv, Wout):
    out, _ = _run(x, Wqkv, Wout)
    return out


# revision 6
# speedup vs baseline: 1.2882x; 1.0246x over previous
"""Trainium2 Bass kernel for nn_MultiHeadAttention_63015760167496.

Computation (see reference): qkv = x @ Wqkv; RoPE on q,k; causal softmax
attention per head; out = einsum('bhts,bshd->bhtd', probs, v);
out.reshape(B,T,C) @ Wout  -- the reshape is a *head-major* flatten of
[B,H,T,D] into [B,T,C], so final-output row r = h*128 + t//16 depends only
on head h.  Sharding: head-parallel over 8 cores (2 heads/core); every core
computes its two heads end-to-end and produces final-output rows
[256*i, 256*i+256).  Host concatenates -- no collectives.

All on-device data is bf16 (PSUM accumulation f32), which halves DMA/SBUF
vs f32r at the same 1 cycle/row PE rate.  Attention runs in S^T layout
([s,t]): softmax denominator via a ones-column matmul (partition reduction
on the PE), normalization broadcast via gpsimd.partition_broadcast and a
flat VE multiply, both issued *deferred* (inside the next t-tile's block
loop) so they never gate the PE.  The attention inner loop is
software-pipelined: score matmuls run LOOKAHEAD blocks ahead of the PV/sum
matmuls so the PE never stalls on the Act-engine exp.  O^T is stored flat
[d, t]; the out-projection reads it through a strided LDWEIGHTS view.
Out-projection chains for head 0 are interleaved into head 1's attention
to fill PE gaps.  Wout is prefetched into SBUF during attention b=0.
"""

import math
import sys

for _p in ("/opt/trn_rl_repo", "/root/.axon_site/_ro/trn_rl_repo"):
    if _p not in sys.path:
        sys.path.insert(0, _p)

import numpy as np
import ml_dtypes

import concourse.bass as bass
import concourse.mybir as mybir
import concourse.tile as tile
from concourse import bacc
from concourse.bass_utils import run_bass_kernel_spmd

B, T, C = 2, 2048, 2048
H = 16            # heads total
D = C // H        # 128 head dim
HALF = D // 2     # 64
P = 128
KO = C // P       # 16 contraction chunks
NCORES = 8
HPC = H // NCORES  # 2 heads per core
TQ = 512          # t-tile for qkv projection
NT = T // TQ      # 4
TA = 512          # t-tile for attention
NTA = T // TA     # 4
NSC = T // P      # 16 s-chunks
ROPE_BASE = 10000.0
SCALE = 1.0 / math.sqrt(D)
TC_ = 512         # col-tile for out projection
NCP = C // TC_    # 4
LOOKAHEAD = 2     # attention software-pipeline depth

f32 = mybir.dt.float32
bf16 = mybir.dt.bfloat16
EXP = mybir.ActivationFunctionType.Exp


def _build():
    nc = bacc.Bacc("TRN2", target_bir_lowering=False, debug=False,
                   num_devices=NCORES)

    # host-pre-tiled x^T: xTt[b, ti, p, ko, u] = x[b, ti*TQ+u, ko*128+p]
    xTt = nc.dram_tensor("xTt", [B, NT, P, KO, TQ], bf16, kind="ExternalInput")
    # host-pre-chunked weights: w[p, ko, m] = W[ko*128+p, m]
    wq = nc.dram_tensor("wq", [P, KO, HPC * D], bf16, kind="ExternalInput")
    wk = nc.dram_tensor("wk", [P, KO, HPC * D], bf16, kind="ExternalInput")
    wv = nc.dram_tensor("wv", [P, KO, HPC * D], bf16, kind="ExternalInput")
    # woutT[j, p, c] = Wout[j*128+p, c]
    woutT = nc.dram_tensor("woutT", [KO, P, C], bf16, kind="ExternalInput")
    cs2 = nc.dram_tensor("cs2", [P, T], bf16, kind="ExternalInput")  # [cos;cos]
    sn1 = nc.dram_tensor("sn1", [HALF, T], bf16, kind="ExternalInput")  # sin
    maskM = nc.dram_tensor("maskM", [P, P], bf16, kind="ExternalInput")
    y = nc.dram_tensor("y", [B, HPC * D, C], f32, kind="ExternalOutput")

    with tile.TileContext(nc) as tc:
        with tc.tile_pool(name="const", bufs=1) as cp_, \
             tc.tile_pool(name="wo", bufs=1) as wop, \
             tc.tile_pool(name="qkv", bufs=1) as qp, \
             tc.tile_pool(name="ot", bufs=1) as op_, \
             tc.tile_pool(name="ys", bufs=3) as yp, \
             tc.tile_pool(name="small", bufs=2) as sp:

            wq_sb = cp_.tile([P, KO, HPC * D], bf16, tag="wq")
            wk_sb = cp_.tile([P, KO, HPC * D], bf16, tag="wk")
            wv_sb = cp_.tile([P, KO, HPC * D], bf16, tag="wv")
            cs_sb = cp_.tile([P, T], bf16, tag="cs")
            sn_sb = cp_.tile([HALF, T], bf16, tag="sn")
            mask_sb = cp_.tile([P, P], bf16, tag="mask")
            wout_sb = wop.tile([P, KO, C], bf16, tag="wout")

            # startup DMAs: wq first (chunked) so the first chain starts
            # ASAP; everything else ordered by first-use time.
            for g in range(2):
                nc.sync.dma_start(wq_sb[:, 8 * g:8 * g + 8, :],
                                  wq.ap()[:, 8 * g:8 * g + 8, :])

            ones_f32 = cp_.tile([P, 1], f32, tag="ones_f32")
            nc.vector.memset(ones_f32[:], 1.0)
            ones_col = cp_.tile([P, 1], bf16, tag="ones_col")
            nc.vector.tensor_copy(ones_col[:], ones_f32[:])
            # act-table warmup: force the Exp table load at t=0 instead of
            # in the middle of the first attention block.
            warm_in = cp_.tile([1, 8], f32, tag="warm_in")
            nc.vector.memset(warm_in[:], 0.0)
            warm_out = cp_.tile([1, 8], f32, tag="warm_out")
            nc.scalar.activation(warm_out[:], warm_in[:], EXP, scale=1.0)

            # persistent attention outputs O^T per (b, local head): [d, t]
            oT = [[op_.tile([P, T], bf16, tag=f"oT{b}{hh}", name=f"oT{b}{hh}")
                   for hh in range(HPC)] for b in range(B)]

            def outproj_chain(b, hh, cpi):
                csl = slice(cpi * TC_, (cpi + 1) * TC_)
                psy = psc_pool[0].tile([P, TC_], f32, tag="y")
                # stationary: oT columns {t : t%16 == j}, strided view
                ovw = oT[b][hh].rearrange("p (u j) -> p j u", j=KO)
                for j in range(KO):
                    nc.tensor.matmul(psy[:], ovw[:, j, :],
                                     wout_sb[:, j, csl],
                                     start=(j == 0), stop=(j == KO - 1))
                ysb = yp.tile([P, TC_], f32, tag="ysb")
                nc.scalar.copy(ysb[:], psy[:])
                nc.sync.dma_start(
                    y.ap()[b, hh * D:(hh + 1) * D, csl], ysb[:])

            psc_pool = [None]

            for b in range(B):
                qT = [qp.tile([P, T], bf16, tag=f"qT{hh}", name=f"qT{b}{hh}")
                      for hh in range(HPC)]
                kT = [qp.tile([P, T], bf16, tag=f"kT{hh}", name=f"kT{b}{hh}")
                      for hh in range(HPC)]
                vt = [qp.tile([P, NSC, D], bf16, tag=f"v{hh}", name=f"v{b}{hh}")
                      for hh in range(HPC)]

                # ---------------- QKV projection + RoPE ----------------
                with tc.tile_pool(name=f"xt{b}", bufs=2) as xp, \
                     tc.tile_pool(name=f"psA{b}", bufs=3, space="PSUM") as psa, \
                     tc.tile_pool(name=f"psV{b}", bufs=2, space="PSUM") as psv_p, \
                     tc.tile_pool(name=f"rope{b}", bufs=3) as rp:

                    def qkmm(xt, w_sb, hh, nm):
                        hsl = slice(hh * D, (hh + 1) * D)
                        ps = psa.tile([P, TQ], f32, tag="acc", name=nm)
                        for ko in range(KO):
                            nc.tensor.matmul(ps[:], w_sb[:, ko, hsl],
                                             xt[:, ko, :],
                                             start=(ko == 0),
                                             stop=(ko == KO - 1))
                        return ps

                    def rope(ps, dst, sl):
                        # tcos = ps * [cos;cos]; tsw pre-swaps halves:
                        # tsw[0:64]=q2*sin, tsw[64:128]=q1*sin so the add/sub
                        # reads align on base partitions.  All elementwise
                        # work on the VE (bf16 operands get 2x mode).
                        cs = cs_sb[:, sl]
                        sn = sn_sb[:, sl]
                        tcos = rp.tile([P, TQ], bf16, tag="tcos")
                        tsw = rp.tile([P, TQ], bf16, tag="tsw")
                        nc.vector.tensor_mul(tcos[:], ps[:], cs)
                        nc.vector.tensor_mul(tsw[0:HALF, :], ps[HALF:P, :], sn)
                        nc.vector.tensor_mul(tsw[HALF:P, :], ps[0:HALF, :], sn)
                        nc.vector.tensor_sub(dst[0:HALF, sl],
                                             tcos[0:HALF, :], tsw[0:HALF, :])
                        nc.vector.tensor_add(dst[HALF:P, sl],
                                             tcos[HALF:P, :], tsw[HALF:P, :])

                    def vchain(xt, ti):
                        for sub in range(TQ // P):
                            psv = psv_p.tile([P, HPC * D], f32, tag="acc")
                            for ko in range(KO):
                                nc.tensor.matmul(
                                    psv[:], xt[:, ko, sub * P:(sub + 1) * P],
                                    wv_sb[:, ko, :],
                                    start=(ko == 0), stop=(ko == KO - 1))
                            tci = ti * (TQ // P) + sub
                            for hh in range(HPC):
                                # Act engine is idle during QKV; it does the
                                # psum->sbuf v copies.
                                nc.scalar.copy(
                                    vt[hh][:, tci, :],
                                    psv[:, hh * D:(hh + 1) * D])

                    xts = {}
                    for ti in range(NT):
                        xts[ti] = xp.tile([P, KO, TQ], bf16, tag="xt",
                                          name=f"xt{b}_{ti}")

                    if b == 0:
                        # Startup is a DMA-bandwidth wall: ~7MB must land in
                        # the first ~30us.  Chunk the first two x tiles so
                        # chains pace behind arriving data, order DMAs by
                        # first-use time, and defer ti0's v-chains until
                        # after ti1's q/k so wv is needed later.
                        for g in range(4):
                            nc.sync.dma_start(
                                xts[0][:, 4 * g:4 * g + 4, :],
                                xTt.ap()[b, 0, :, 4 * g:4 * g + 4, :])
                        ps = qkmm(xts[0], wq_sb, 0, "acc0_q0")
                        nc.sync.dma_start(wk_sb[:], wk.ap())
                        nc.sync.dma_start(cs_sb[:], cs2.ap())
                        nc.sync.dma_start(sn_sb[:], sn1.ap())
                        rope(ps, qT[0], slice(0, TQ))
                        rope(qkmm(xts[0], wq_sb, 1, "acc0_q1"), qT[1],
                             slice(0, TQ))
                        for g in range(4):
                            nc.sync.dma_start(
                                xts[1][:, 4 * g:4 * g + 4, :],
                                xTt.ap()[b, 1, :, 4 * g:4 * g + 4, :])
                        rope(qkmm(xts[0], wk_sb, 0, "acc0_k0"), kT[0],
                             slice(0, TQ))
                        nc.sync.dma_start(wv_sb[:], wv.ap())
                        nc.sync.dma_start(mask_sb[:], maskM.ap())
                        rope(qkmm(xts[0], wk_sb, 1, "acc0_k1"), kT[1],
                             slice(0, TQ))
                        sl1 = slice(TQ, 2 * TQ)
                        rope(qkmm(xts[1], wq_sb, 0, "acc1_q0"), qT[0], sl1)
                        rope(qkmm(xts[1], wq_sb, 1, "acc1_q1"), qT[1], sl1)
                        rope(qkmm(xts[1], wk_sb, 0, "acc1_k0"), kT[0], sl1)
                        rope(qkmm(xts[1], wk_sb, 1, "acc1_k1"), kT[1], sl1)
                        vchain(xts[1], 1)
                        vchain(xts[0], 0)
                        rest = range(2, NT)
                    else:
                        rest = range(NT)

                    for ti in rest:
                        sl = slice(ti * TQ, (ti + 1) * TQ)
                        xt = xts[ti]
                        nc.sync.dma_start(xt[:], xTt.ap()[b, ti])
                        for hh in range(HPC):
                            rope(qkmm(xt, wq_sb, hh, f"a{ti}q{hh}"),
                                 qT[hh], sl)
                            rope(qkmm(xt, wk_sb, hh, f"a{ti}k{hh}"),
                                 kT[hh], sl)
                        vchain(xt, ti)

                # ------------- attention (S^T layout) + interleaved -----
                # ------------- out-projection of the previous head ------
                with tc.tile_pool(name=f"psBsc{b}", bufs=3, space="PSUM") as pssc, \
                     tc.tile_pool(name=f"psBo{b}", bufs=2, space="PSUM") as pso, \
                     tc.tile_pool(name=f"psBsum{b}", bufs=2, space="PSUM") as pssum, \
                     tc.tile_pool(name=f"psC{b}", bufs=1, space="PSUM") as psc, \
                     tc.tile_pool(name=f"pt{b}", bufs=LOOKAHEAD + 2) as ptp:
                    psc_pool[0] = psc
                    nwo = 0   # wout prefetch cursor (b == 0 only)
                    # Deferred-issue queues.  norm_q drains exactly one
                    # t-tile later (ps_o/ps_sum pools have bufs=2, so the
                    # normalize MUST be issued before the slot cycles);
                    # op_q (out-projection chains) drains one per t-tile.
                    norm_q = []
                    op_q = []

                    for hh in range(HPC):
                        for ta in range(NTA):
                            spt = TA // P
                            tsl = slice(ta * TA, (ta + 1) * TA)
                            ps_o = pso.tile([P, TA], f32, tag="o")
                            ps_sum = pssum.tile([1, TA], f32, tag="sum")
                            smax = (ta + 1) * spt - 1
                            pend = []

                            def flush(last):
                                pt_, w_, s_, first_ = pend.pop(0)
                                nc.tensor.matmul(ps_o[:, w_],
                                                 vt[hh][:, s_, :],
                                                 pt_[:, w_],
                                                 start=first_, stop=last)
                                nc.tensor.matmul(ps_sum[:, w_], ones_col[:],
                                                 pt_[:, w_],
                                                 start=first_, stop=last)

                            for s in range(smax + 1):
                                diag = s >= ta * spt
                                t_lo = (s - ta * spt) * P if diag else 0
                                w = slice(t_lo, TA)
                                qsl = slice(ta * TA + t_lo, (ta + 1) * TA)
                                ps_sc = pssc.tile([P, TA], f32, tag="sc")
                                nc.tensor.matmul(
                                    ps_sc[:, w],
                                    kT[hh][:, s * P:(s + 1) * P],
                                    qT[hh][:, qsl], start=True, stop=True)
                                pt = ptp.tile([P, TA], bf16, tag="pt")
                                nc.scalar.activation(pt[:, w], ps_sc[:, w],
                                                     EXP, scale=SCALE)
                                if diag:  # mask the 128x128 triangle
                                    nc.vector.tensor_mul(
                                        pt[:, t_lo:t_lo + P],
                                        pt[:, t_lo:t_lo + P], mask_sb[:])
                                pend.append((pt, w, s, s == 0))
                                if len(pend) > LOOKAHEAD:
                                    flush(False)
                                if s == 1 and norm_q:
                                    norm_q.pop(0)()
                                if s == 3 and op_q:
                                    op_q.pop(0)()
                            while len(pend) > 1:
                                flush(False)
                            flush(True)

                            # normalization, deferred: recip on VE,
                            # partition-broadcast on gpsimd, flat multiply
                            # on VE -- issued inside the next tile's block
                            # loop so the PE/Act pipeline never waits.
                            def normalize(ps_o=ps_o, ps_sum=ps_sum,
                                          hh=hh, tsl=tsl):
                                recf = sp.tile([1, TA], f32, tag="recf")
                                nc.vector.reciprocal_approx_fast(recf[:],
                                                                 ps_sum[:])
                                recb = sp.tile([1, TA], bf16, tag="recb")
                                nc.vector.tensor_copy(recb[:], recf[:])
                                bcb = sp.tile([P, TA], bf16, tag="bcb")
                                nc.gpsimd.partition_broadcast(bcb[:],
                                                              recb[:],
                                                              channels=P)
                                nc.vector.tensor_mul(oT[b][hh][:, tsl],
                                                     ps_o[:], bcb[:])
                            norm_q.append(normalize)

                            if b == 0 and hh == 0:
                                # prefetch all of wout during head 0's
                                # attention (the first out-proj chain needs
                                # every j block)
                                for _ in range(4):
                                    nc.sync.dma_start(
                                        wout_sb[:, nwo, :], woutT.ap()[nwo])
                                    nwo += 1

                        if hh == 0:
                            # out-projection chains of head 0 interleave
                            # into head 1's attention
                            for cpi in range(NCP):
                                op_q.append(
                                    lambda cpi=cpi: outproj_chain(b, 0, cpi))
                    for t_ in norm_q:
                        t_()
                    for t_ in op_q:
                        t_()
                    for cpi in range(NCP):
                        outproj_chain(b, 1, cpi)

    nc.compile()
    return nc


_NC = None


def _get_nc():
    global _NC
    if _NC is None:
        _NC = _build()
    return _NC


def _host_tables():
    pos = np.arange(T, dtype=np.float32)[:, None]
    div = np.exp(np.arange(0, 2 * HALF, 2, dtype=np.float32)
                 * np.float32(-math.log(ROPE_BASE) / (2 * HALF)))
    ang = pos * div[None, :]
    cosv = np.cos(ang).astype(np.float32)   # [T, HALF]
    sinv = np.sin(ang).astype(np.float32)
    cosT = np.ascontiguousarray(cosv.T)     # [HALF, T]
    sinT = np.ascontiguousarray(sinv.T)
    cs2 = np.ascontiguousarray(
        np.concatenate([cosT, cosT], axis=0)).astype(ml_dtypes.bfloat16)
    sn1 = np.ascontiguousarray(sinT).astype(ml_dtypes.bfloat16)
    # triangle mask M[s, w] = 1 iff s <= w
    ww = np.arange(P)[None, :]
    ss = np.arange(P)[:, None]
    maskM = (ss <= ww).astype(ml_dtypes.bfloat16)
    return cs2, sn1, maskM


def _make_in_maps(x, Wqkv, Wout):
    x = np.asarray(x, dtype=np.float32)
    Wqkv = np.asarray(Wqkv, dtype=np.float32)
    Wout = np.asarray(Wout, dtype=np.float32)
    assert x.shape == (B, T, C) and Wqkv.shape == (C, 3 * C) \
        and Wout.shape == (C, C)

    cs2, sn1, maskM = _host_tables()
    # xTt[b, ti, p, ko, u] = x[b, ti*TQ+u, ko*128+p]
    xTt = np.ascontiguousarray(
        x.reshape(B, NT, TQ, KO, P).transpose(0, 1, 4, 3, 2)
    ).astype(ml_dtypes.bfloat16)
    woutT = np.ascontiguousarray(
        Wout.reshape(KO, P, C)).astype(ml_dtypes.bfloat16)

    in_maps = []
    for core in range(NCORES):
        h0 = core * HPC
        cols = slice(h0 * D, (h0 + HPC) * D)
        ws = []
        for part in range(3):
            w = Wqkv[:, part * C:(part + 1) * C][:, cols]  # [C, HPC*D]
            ws.append(np.ascontiguousarray(
                w.reshape(KO, P, HPC * D).transpose(1, 0, 2)
            ).astype(ml_dtypes.bfloat16))
        in_maps.append({
            "xTt": xTt,
            "wq": ws[0], "wk": ws[1], "wv": ws[2],
            "woutT": woutT,
            "cs2": cs2, "sn1": sn1, "maskM": maskM,
        })
    return in_maps


def _run(x, Wqkv, Wout, trace=False):
    nc = _get_nc()
    in_maps = _make_in_maps(x, Wqkv, Wout)
    res = run_bass_kernel_spmd(nc, in_maps, core_ids=list(range(NCORES)),
                               trace=trace)
    out = np.empty((B, T, C), dtype=np.float32)
    for core in range(NCORES):
        out[:, core * HPC * D:(core + 1) * HPC * D, :] = \
            res.results[core]["y"]
    return out, res


def kernel(x, Wqkv, Wout):
    out, _ = _run(x, Wqkv, Wout)
    return out


# revision 18
# speedup vs baseline: 1.3572x; 1.0536x over previous
"""Trainium2 Bass kernel for nn_MultiHeadAttention_63015760167496.

Computation (see reference): qkv = x @ Wqkv; RoPE on q,k; causal softmax
attention per head; out = einsum('bhts,bshd->bhtd', probs, v);
out.reshape(B,T,C) @ Wout  -- the reshape is a *head-major* flatten of
[B,H,T,D] into [B,T,C], so final-output row r = h*128 + t//16 depends only
on head h.  Sharding: head-parallel over 8 cores (2 heads/core); every core
computes its two heads end-to-end and produces final-output rows
[256*i, 256*i+256).  Host concatenates -- no collectives.

All on-device data is bf16 (PSUM accumulation f32), which halves DMA/SBUF
vs f32r at the same 1 cycle/row PE rate.  Attention runs in S^T layout
([s,t]): softmax denominator via a ones-column matmul (partition reduction
on the PE), normalization broadcast via gpsimd.partition_broadcast and a
flat VE multiply, both issued *deferred* (inside the next t-tile's block
loop) so they never gate the PE.  The attention inner loop is
software-pipelined: score matmuls run LOOKAHEAD blocks ahead of the PV/sum
matmuls so the PE never stalls on the Act-engine exp.  O^T is stored flat
[d, t]; the out-projection reads it through a strided LDWEIGHTS view.
Out-projection chains for head 0 are interleaved into head 1's attention
to fill PE gaps.  Wout is prefetched into SBUF during attention b=0.
"""

import math
import sys

for _p in ("/opt/trn_rl_repo", "/root/.axon_site/_ro/trn_rl_repo"):
    if _p not in sys.path:
        sys.path.insert(0, _p)

import numpy as np
import ml_dtypes

import concourse.bass as bass
import concourse.mybir as mybir
import concourse.tile as tile
from concourse import bacc
from concourse.bass_utils import run_bass_kernel_spmd

B, T, C = 2, 2048, 2048
H = 16            # heads total
D = C // H        # 128 head dim
HALF = D // 2     # 64
P = 128
KO = C // P       # 16 contraction chunks
NCORES = 8
HPC = H // NCORES  # 2 heads per core
TQ = 512          # t-tile for qkv projection
NT = T // TQ      # 4
TA = 512          # t-tile for attention
NTA = T // TA     # 4
NSC = T // P      # 16 s-chunks
ROPE_BASE = 10000.0
SCALE = 1.0 / math.sqrt(D)
TC_ = 512         # col-tile for out projection
NCP = C // TC_    # 4
LOOKAHEAD = 2     # attention software-pipeline depth

f32 = mybir.dt.float32
bf16 = mybir.dt.bfloat16
EXP = mybir.ActivationFunctionType.Exp


def _build():
    nc = bacc.Bacc("TRN2", target_bir_lowering=False, debug=False,
                   num_devices=NCORES)

    # host-pre-tiled x^T: xTt[b, ti, p, ko, u] = x[b, ti*TQ+u, ko*128+p]
    xTt = nc.dram_tensor("xTt", [B, NT, P, KO, TQ], bf16, kind="ExternalInput")
    # host-pre-chunked weights: w[p, ko, m] = W[ko*128+p, m]
    wq = nc.dram_tensor("wq", [P, KO, HPC * D], bf16, kind="ExternalInput")
    wk = nc.dram_tensor("wk", [P, KO, HPC * D], bf16, kind="ExternalInput")
    wv = nc.dram_tensor("wv", [P, KO, HPC * D], bf16, kind="ExternalInput")
    # woutT[j, p, c] = Wout[j*128+p, c]
    woutT = nc.dram_tensor("woutT", [KO, P, C], bf16, kind="ExternalInput")
    cs2 = nc.dram_tensor("cs2", [P, T], bf16, kind="ExternalInput")  # [cos;cos]
    sn1 = nc.dram_tensor("sn1", [HALF, T], bf16, kind="ExternalInput")  # sin
    maskM = nc.dram_tensor("maskM", [P, P], bf16, kind="ExternalInput")
    y = nc.dram_tensor("y", [B, HPC * D, C], f32, kind="ExternalOutput")

    with tile.TileContext(nc) as tc:
        with tc.tile_pool(name="const", bufs=1) as cp_, \
             tc.tile_pool(name="wo", bufs=1) as wop, \
             tc.tile_pool(name="qkv", bufs=1) as qp, \
             tc.tile_pool(name="ot", bufs=1) as op_, \
             tc.tile_pool(name="ys", bufs=3) as yp, \
             tc.tile_pool(name="small", bufs=2) as sp:

            wq_sb = cp_.tile([P, KO, HPC * D], bf16, tag="wq")
            wk_sb = cp_.tile([P, KO, HPC * D], bf16, tag="wk")
            wv_sb = cp_.tile([P, KO, HPC * D], bf16, tag="wv")
            cs_sb = cp_.tile([P, T], bf16, tag="cs")
            sn_sb = cp_.tile([HALF, T], bf16, tag="sn")
            mask_sb = cp_.tile([P, P], bf16, tag="mask")
            wout_sb = wop.tile([P, KO, C], bf16, tag="wout")

            # startup DMAs: wq first (chunked) so the first chain starts
            # ASAP; everything else ordered by first-use time.  The gpsimd
            # engine finishes its framework preamble ~2.5us before the sync
            # engine, so the very first chunks go through its DGE.
            nc.sync.dma_start(wq_sb[:, 0:8, :], wq.ap()[:, 0:8, :])
            nc.sync.dma_start(wq_sb[:, 8:16, :], wq.ap()[:, 8:16, :])

            ones_f32 = cp_.tile([P, 1], f32, tag="ones_f32")
            nc.vector.memset(ones_f32[:], 1.0)
            ones_col = cp_.tile([P, 1], bf16, tag="ones_col")
            nc.vector.tensor_copy(ones_col[:], ones_f32[:])
            # act-table warmup: force the Exp table load at t=0 instead of
            # in the middle of the first attention block.
            warm_in = cp_.tile([1, 8], f32, tag="warm_in")
            nc.vector.memset(warm_in[:], 0.0)
            warm_out = cp_.tile([1, 8], f32, tag="warm_out")
            nc.scalar.activation(warm_out[:], warm_in[:], EXP, scale=1.0)

            # persistent attention outputs O^T per (b, local head): [d, t]
            oT = [[op_.tile([P, T], bf16, tag=f"oT{b}{hh}", name=f"oT{b}{hh}")
                   for hh in range(HPC)] for b in range(B)]

            def outproj_chain(b, hh, cpi):
                csl = slice(cpi * TC_, (cpi + 1) * TC_)
                psy = psc_pool[0].tile([P, TC_], f32, tag="y")
                # stationary: oT columns {t : t%16 == j}, strided view
                ovw = oT[b][hh].rearrange("p (u j) -> p j u", j=KO)
                for j in range(KO):
                    nc.tensor.matmul(psy[:], ovw[:, j, :],
                                     wout_sb[:, j, csl],
                                     start=(j == 0), stop=(j == KO - 1))
                ysb = yp.tile([P, TC_], f32, tag="ysb")
                nc.scalar.copy(ysb[:], psy[:])
                nc.sync.dma_start(
                    y.ap()[b, hh * D:(hh + 1) * D, csl], ysb[:])

            psc_pool = [None]

            for b in range(B):
                qT = [qp.tile([P, T], bf16, tag=f"qT{hh}", name=f"qT{b}{hh}")
                      for hh in range(HPC)]
                kT = [qp.tile([P, T], bf16, tag=f"kT{hh}", name=f"kT{b}{hh}")
                      for hh in range(HPC)]
                vt = [qp.tile([P, NSC, D], bf16, tag=f"v{hh}", name=f"v{b}{hh}")
                      for hh in range(HPC)]

                # ---------------- QKV projection + RoPE ----------------
                with tc.tile_pool(name=f"xt{b}", bufs=2) as xp, \
                     tc.tile_pool(name=f"psA{b}", bufs=3, space="PSUM") as psa, \
                     tc.tile_pool(name=f"psV{b}", bufs=2, space="PSUM") as psv_p, \
                     tc.tile_pool(name=f"rope{b}", bufs=3) as rp:

                    def qkmm(xt, w_sb, hh, nm):
                        hsl = slice(hh * D, (hh + 1) * D)
                        ps = psa.tile([P, TQ], f32, tag="acc", name=nm)
                        for ko in range(KO):
                            nc.tensor.matmul(ps[:], w_sb[:, ko, hsl],
                                             xt[:, ko, :],
                                             start=(ko == 0),
                                             stop=(ko == KO - 1))
                        return ps

                    def rope(ps, dst, sl):
                        # tcos = ps * [cos;cos]; tsw pre-swaps halves:
                        # tsw[0:64]=q2*sin, tsw[64:128]=q1*sin so the add/sub
                        # reads align on base partitions.  All elementwise
                        # work on the VE (bf16 operands get 2x mode).
                        cs = cs_sb[:, sl]
                        sn = sn_sb[:, sl]
                        tcos = rp.tile([P, TQ], bf16, tag="tcos")
                        tsw = rp.tile([P, TQ], bf16, tag="tsw")
                        nc.vector.tensor_mul(tcos[:], ps[:], cs)
                        nc.vector.tensor_mul(tsw[0:HALF, :], ps[HALF:P, :], sn)
                        nc.vector.tensor_mul(tsw[HALF:P, :], ps[0:HALF, :], sn)
                        nc.vector.tensor_sub(dst[0:HALF, sl],
                                             tcos[0:HALF, :], tsw[0:HALF, :])
                        nc.vector.tensor_add(dst[HALF:P, sl],
                                             tcos[HALF:P, :], tsw[HALF:P, :])

                    def vchain(xt, ti):
                        for sub in range(TQ // P):
                            psv = psv_p.tile([P, HPC * D], f32, tag="acc")
                            for ko in range(KO):
                                nc.tensor.matmul(
                                    psv[:], xt[:, ko, sub * P:(sub + 1) * P],
                                    wv_sb[:, ko, :],
                                    start=(ko == 0), stop=(ko == KO - 1))
                            tci = ti * (TQ // P) + sub
                            for hh in range(HPC):
                                # Act engine is idle during QKV; it does the
                                # psum->sbuf v copies.
                                nc.scalar.copy(
                                    vt[hh][:, tci, :],
                                    psv[:, hh * D:(hh + 1) * D])

                    xts = {}
                    for ti in range(NT):
                        xts[ti] = xp.tile([P, KO, TQ], bf16, tag="xt",
                                          name=f"xt{b}_{ti}")

                    if b == 0:
                        # Startup is a DMA-bandwidth wall: ~7MB must land in
                        # the first ~30us.  Chunk the first two x tiles so
                        # chains pace behind arriving data, order DMAs by
                        # first-use time, and defer ti0's v-chains until
                        # after ti1's q/k so wv is needed later.
                        nc.sync.dma_start(xts[0][:, 0:4, :],
                                          xTt.ap()[b, 0, :, 0:4, :])
                        for g in range(1, 4):
                            nc.sync.dma_start(
                                xts[0][:, 4 * g:4 * g + 4, :],
                                xTt.ap()[b, 0, :, 4 * g:4 * g + 4, :])
                        ps = qkmm(xts[0], wq_sb, 0, "acc0_q0")
                        nc.sync.dma_start(wk_sb[:], wk.ap())
                        nc.sync.dma_start(cs_sb[:], cs2.ap())
                        nc.sync.dma_start(sn_sb[:], sn1.ap())
                        rope(ps, qT[0], slice(0, TQ))
                        rope(qkmm(xts[0], wq_sb, 1, "acc0_q1"), qT[1],
                             slice(0, TQ))
                        for g in range(4):
                            nc.sync.dma_start(
                                xts[1][:, 4 * g:4 * g + 4, :],
                                xTt.ap()[b, 1, :, 4 * g:4 * g + 4, :])
                        rope(qkmm(xts[0], wk_sb, 0, "acc0_k0"), kT[0],
                             slice(0, TQ))
                        nc.sync.dma_start(wv_sb[:], wv.ap())
                        nc.sync.dma_start(mask_sb[:], maskM.ap())
                        rope(qkmm(xts[0], wk_sb, 1, "acc0_k1"), kT[1],
                             slice(0, TQ))
                        sl1 = slice(TQ, 2 * TQ)
                        rope(qkmm(xts[1], wq_sb, 0, "acc1_q0"), qT[0], sl1)
                        rope(qkmm(xts[1], wq_sb, 1, "acc1_q1"), qT[1], sl1)
                        rope(qkmm(xts[1], wk_sb, 0, "acc1_k0"), kT[0], sl1)
                        rope(qkmm(xts[1], wk_sb, 1, "acc1_k1"), kT[1], sl1)
                        vchain(xts[1], 1)
                        vchain(xts[0], 0)
                        rest = range(2, NT)
                    else:
                        rest = range(NT)

                    for ti in rest:
                        sl = slice(ti * TQ, (ti + 1) * TQ)
                        xt = xts[ti]
                        nc.sync.dma_start(xt[:], xTt.ap()[b, ti])
                        for hh in range(HPC):
                            rope(qkmm(xt, wq_sb, hh, f"a{ti}q{hh}"),
                                 qT[hh], sl)
                            rope(qkmm(xt, wk_sb, hh, f"a{ti}k{hh}"),
                                 kT[hh], sl)
                        vchain(xt, ti)

                # ------------- attention (S^T layout) + interleaved -----
                # ------------- out-projection of the previous head ------
                # s-chunks are processed in PAIRS sharing one 2-bank PSUM
                # tile and a single exp instruction, so the Act engine
                # (1024 cols + one fixed overhead) runs faster than the
                # PE's 6 matmuls per pair and never paces the pipeline.
                with tc.tile_pool(name=f"psBsc{b}", bufs=2, space="PSUM") as pssc, \
                     tc.tile_pool(name=f"psBo{b}", bufs=2, space="PSUM") as pso, \
                     tc.tile_pool(name=f"psBsum{b}", bufs=2, space="PSUM") as pssum, \
                     tc.tile_pool(name=f"pt{b}", bufs=3) as ptp:
                    nwo = 0   # wout prefetch cursor (b == 0 only)
                    # Deferred-issue queues.  norm_q drains exactly one
                    # t-tile later (ps_o has bufs=2 and the sum bank has
                    # two parity slots, so the normalize MUST be issued
                    # before the slot cycles); op_q drains one per t-tile.
                    norm_q = []
                    op_q = []

                    for hh in range(HPC):
                        for ta in range(NTA):
                            spt = TA // P
                            tsl = slice(ta * TA, (ta + 1) * TA)
                            ps_o = pso.tile([P, TA], f32, tag="o")
                            ps_sum = pssum.tile([1, TA], f32, tag="sum")
                            nblk = (ta + 1) * spt
                            pend = []

                            def flush(last):
                                # same-accumulation-group matmuls must be
                                # adjacent: a LDWEIGHTS that follows an
                                # accumulating matmul whose group is being
                                # suspended stalls ~95ns on hw, so issue
                                # [oA, oB] then [sumA, sumB], not
                                # interleaved.
                                pt_, sws = pend.pop(0)
                                for k, (s_, w_) in enumerate(sws):
                                    nc.tensor.matmul(ps_o[:, w_],
                                                     vt[hh][:, s_, :],
                                                     pt_[:, k, w_],
                                                     start=(s_ == 0),
                                                     stop=(last and
                                                           k == len(sws) - 1))
                                for k, (s_, w_) in enumerate(sws):
                                    nc.tensor.matmul(
                                        ps_sum[:, w_],
                                        ones_col[:], pt_[:, k, w_],
                                        start=(s_ == 0),
                                        stop=(last and k == len(sws) - 1))

                            for pi in range(nblk // 2):
                                ps_sc = pssc.tile([P, 2, TA], f32, tag="sc")
                                pt = ptp.tile([P, 2, TA], bf16, tag="pt")
                                sws = []
                                for k in range(2):
                                    s = 2 * pi + k
                                    diag = s >= ta * spt
                                    t_lo = (s - ta * spt) * P if diag else 0
                                    w = slice(t_lo, TA)
                                    qsl = slice(ta * TA + t_lo,
                                                (ta + 1) * TA)
                                    nc.tensor.matmul(
                                        ps_sc[:, k, w],
                                        kT[hh][:, s * P:(s + 1) * P],
                                        qT[hh][:, qsl],
                                        start=True, stop=True)
                                    sws.append((s, w))
                                # one exp for both chunks; cols outside a
                                # diag chunk's window hold stale psum ->
                                # garbage pt that no matmul reads
                                nc.scalar.activation(pt[:, :, :],
                                                     ps_sc[:, :, :],
                                                     EXP, scale=SCALE)
                                for k, (s, w) in enumerate(sws):
                                    if s >= ta * spt:  # mask the triangle
                                        t_lo = (s - ta * spt) * P
                                        nc.vector.tensor_mul(
                                            pt[:, k, t_lo:t_lo + P],
                                            pt[:, k, t_lo:t_lo + P],
                                            mask_sb[:])
                                pend.append((pt, sws))
                                if len(pend) > 1:
                                    flush(False)
                                if pi == 0 and norm_q:
                                    norm_q.pop(0)()
                            flush(True)

                            # normalization, deferred: recip on VE,
                            # partition-broadcast on gpsimd, flat multiply
                            # on VE -- issued inside the next tile's block
                            # loop so the PE/Act pipeline never waits.
                            def normalize(ps_o=ps_o, ps_sum=ps_sum,
                                          hh=hh, tsl=tsl):
                                recf = sp.tile([1, TA], f32, tag="recf")
                                nc.vector.reciprocal_approx_fast(
                                    recf[:], ps_sum[:])
                                recb = sp.tile([1, TA], bf16, tag="recb")
                                nc.vector.tensor_copy(recb[:], recf[:])
                                bcb = sp.tile([P, TA], bf16, tag="bcb")
                                nc.gpsimd.partition_broadcast(bcb[:],
                                                              recb[:],
                                                              channels=P)
                                nc.vector.tensor_mul(oT[b][hh][:, tsl],
                                                     ps_o[:], bcb[:])
                            norm_q.append(normalize)

                            if b == 0 and hh == 0:
                                # prefetch all of wout during head 0's
                                # attention (the first out-proj chain needs
                                # every j block)
                                for _ in range(4):
                                    nc.sync.dma_start(
                                        wout_sb[:, nwo, :], woutT.ap()[nwo])
                                    nwo += 1

                    for t_ in norm_q:
                        t_()

                # ---------------- output projection (pure PE) ----------
                with tc.tile_pool(name=f"psC{b}", bufs=2, space="PSUM") as psc:
                    psc_pool[0] = psc
                    for hh in range(HPC):
                        for cpi in range(NCP):
                            outproj_chain(b, hh, cpi)

    nc.compile()
    return nc


_NC = None


def _get_nc():
    global _NC
    if _NC is None:
        _NC = _build()
    return _NC


def _host_tables():
    pos = np.arange(T, dtype=np.float32)[:, None]
    div = np.exp(np.arange(0, 2 * HALF, 2, dtype=np.float32)
                 * np.float32(-math.log(ROPE_BASE) / (2 * HALF)))
    ang = pos * div[None, :]
    cosv = np.cos(ang).astype(np.float32)   # [T, HALF]
    sinv = np.sin(ang).astype(np.float32)
    cosT = np.ascontiguousarray(cosv.T)     # [HALF, T]
    sinT = np.ascontiguousarray(sinv.T)
    cs2 = np.ascontiguousarray(
        np.concatenate([cosT, cosT], axis=0)).astype(ml_dtypes.bfloat16)
    sn1 = np.ascontiguousarray(sinT).astype(ml_dtypes.bfloat16)
    # triangle mask M[s, w] = 1 iff s <= w
    ww = np.arange(P)[None, :]
    ss = np.arange(P)[:, None]
    maskM = (ss <= ww).astype(ml_dtypes.bfloat16)
    return cs2, sn1, maskM


def _make_in_maps(x, Wqkv, Wout):
    x = np.asarray(x, dtype=np.float32)
    Wqkv = np.asarray(Wqkv, dtype=np.float32)
    Wout = np.asarray(Wout, dtype=np.float32)
    assert x.shape == (B, T, C) and Wqkv.shape == (C, 3 * C) \
        and Wout.shape == (C, C)

    cs2, sn1, maskM = _host_tables()
    # xTt[b, ti, p, ko, u] = x[b, ti*TQ+u, ko*128+p]
    xTt = np.ascontiguousarray(
        x.reshape(B, NT, TQ, KO, P).transpose(0, 1, 4, 3, 2)
    ).astype(ml_dtypes.bfloat16)
    woutT = np.ascontiguousarray(
        Wout.reshape(KO, P, C)).astype(ml_dtypes.bfloat16)

    in_maps = []
    for core in range(NCORES):
        h0 = core * HPC
        cols = slice(h0 * D, (h0 + HPC) * D)
        ws = []
        for part in range(3):
            w = Wqkv[:, part * C:(part + 1) * C][:, cols]  # [C, HPC*D]
            ws.append(np.ascontiguousarray(
                w.reshape(KO, P, HPC * D).transpose(1, 0, 2)
            ).astype(ml_dtypes.bfloat16))
        in_maps.append({
            "xTt": xTt,
            "wq": ws[0], "wk": ws[1], "wv": ws[2],
            "woutT": woutT,
            "cs2": cs2, "sn1": sn1, "maskM": maskM,
        })
    return in_maps


def _run(x, Wqkv, Wout, trace=False):
    nc = _get_nc()
    in_maps = _make_in_maps(x, Wqkv, Wout)
    res = run_bass_kernel_spmd(nc, in_maps, core_ids=list(range(NCORES)),
                               trace=trace)
    out = np.empty((B, T, C), dtype=np.float32)
    for core in range(NCORES):
        out[:, core * HPC * D:(core + 1) * HPC * D, :] = \
            res.results[core]["y"]
    return out, res


def kernel(x, Wqkv, Wout):
    out, _ = _run(x, Wqkv, Wout)
    return out


# revision 22
# speedup vs baseline: 1.3630x; 1.0043x over previous
"""Trainium2 Bass kernel for nn_MultiHeadAttention_63015760167496.

Computation (see reference): qkv = x @ Wqkv; RoPE on q,k; causal softmax
attention per head; out = einsum('bhts,bshd->bhtd', probs, v);
out.reshape(B,T,C) @ Wout  -- the reshape is a *head-major* flatten of
[B,H,T,D] into [B,T,C], so final-output row r = h*128 + t//16 depends only
on head h.  Sharding: head-parallel over 8 cores (2 heads/core); every core
computes its two heads end-to-end and produces final-output rows
[256*i, 256*i+256).  Host concatenates -- no collectives.

All on-device data is bf16 (PSUM accumulation f32), which halves DMA/SBUF
vs f32r at the same 1 cycle/row PE rate.  Attention runs in S^T layout
([s,t]): softmax denominator via a ones-column matmul (partition reduction
on the PE), normalization broadcast via gpsimd.partition_broadcast and a
flat VE multiply, both issued *deferred* (inside the next t-tile's block
loop) so they never gate the PE.  The attention inner loop is
software-pipelined: score matmuls run LOOKAHEAD blocks ahead of the PV/sum
matmuls so the PE never stalls on the Act-engine exp.  O^T is stored flat
[d, t]; the out-projection reads it through a strided LDWEIGHTS view.
Out-projection chains for head 0 are interleaved into head 1's attention
to fill PE gaps.  Wout is prefetched into SBUF during attention b=0.
"""

import math
import sys

for _p in ("/opt/trn_rl_repo", "/root/.axon_site/_ro/trn_rl_repo"):
    if _p not in sys.path:
        sys.path.insert(0, _p)

import numpy as np
import ml_dtypes

import concourse.bass as bass
import concourse.mybir as mybir
import concourse.tile as tile
from concourse import bacc
from concourse.bass_utils import run_bass_kernel_spmd

B, T, C = 2, 2048, 2048
H = 16            # heads total
D = C // H        # 128 head dim
HALF = D // 2     # 64
P = 128
KO = C // P       # 16 contraction chunks
NCORES = 8
HPC = H // NCORES  # 2 heads per core
TQ = 512          # t-tile for qkv projection
NT = T // TQ      # 4
TA = 512          # t-tile for attention
NTA = T // TA     # 4
NSC = T // P      # 16 s-chunks
ROPE_BASE = 10000.0
SCALE = 1.0 / math.sqrt(D)
TC_ = 512         # col-tile for out projection
NCP = C // TC_    # 4
LOOKAHEAD = 2     # attention software-pipeline depth

f32 = mybir.dt.float32
bf16 = mybir.dt.bfloat16
EXP = mybir.ActivationFunctionType.Exp


def _build():
    nc = bacc.Bacc("TRN2", target_bir_lowering=False, debug=False,
                   num_devices=NCORES)

    # host-pre-tiled x^T: xTt[b, ti, p, ko, u] = x[b, ti*TQ+u, ko*128+p]
    xTt = nc.dram_tensor("xTt", [B, NT, P, KO, TQ], bf16, kind="ExternalInput")
    # host-pre-chunked weights: w[p, ko, m] = W[ko*128+p, m]
    wq = nc.dram_tensor("wq", [P, KO, HPC * D], bf16, kind="ExternalInput")
    wk = nc.dram_tensor("wk", [P, KO, HPC * D], bf16, kind="ExternalInput")
    wv = nc.dram_tensor("wv", [P, KO, HPC * D], bf16, kind="ExternalInput")
    # woutT[j, p, c] = Wout[j*128+p, c]
    woutT = nc.dram_tensor("woutT", [KO, P, C], bf16, kind="ExternalInput")
    cs2 = nc.dram_tensor("cs2", [P, T], bf16, kind="ExternalInput")  # [cos;cos]
    sn1 = nc.dram_tensor("sn1", [HALF, T], bf16, kind="ExternalInput")  # sin
    maskM = nc.dram_tensor("maskM", [P, P], bf16, kind="ExternalInput")
    y = nc.dram_tensor("y", [B, HPC * D, C], f32, kind="ExternalOutput")

    with tile.TileContext(nc) as tc:
        with tc.tile_pool(name="const", bufs=1) as cp_, \
             tc.tile_pool(name="wo", bufs=1) as wop, \
             tc.tile_pool(name="qkv", bufs=1) as qp, \
             tc.tile_pool(name="ot", bufs=1) as op_, \
             tc.tile_pool(name="ys", bufs=3) as yp, \
             tc.tile_pool(name="small", bufs=2) as sp:

            wq_sb = cp_.tile([P, KO, HPC * D], bf16, tag="wq")
            wk_sb = cp_.tile([P, KO, HPC * D], bf16, tag="wk")
            wv_sb = cp_.tile([P, KO, HPC * D], bf16, tag="wv")
            cs_sb = cp_.tile([P, T], bf16, tag="cs")
            sn_sb = cp_.tile([HALF, T], bf16, tag="sn")
            mask_sb = cp_.tile([P, P], bf16, tag="mask")
            wout_sb = wop.tile([P, KO, C], bf16, tag="wout")

            # startup DMAs: wq first (chunked) so the first chain starts
            # ASAP; everything else ordered by first-use time.  The gpsimd
            # engine finishes its framework preamble ~2.5us before the sync
            # engine, so the very first chunks go through its DGE.
            nc.gpsimd.dma_start(wq_sb[:, 0:8, :], wq.ap()[:, 0:8, :])
            nc.sync.dma_start(wq_sb[:, 8:16, :], wq.ap()[:, 8:16, :])

            ones_f32 = cp_.tile([P, 1], f32, tag="ones_f32")
            nc.vector.memset(ones_f32[:], 1.0)
            ones_col = cp_.tile([P, 1], bf16, tag="ones_col")
            nc.vector.tensor_copy(ones_col[:], ones_f32[:])
            # act-table warmup: force the Exp table load at t=0 instead of
            # in the middle of the first attention block.
            warm_in = cp_.tile([1, 8], f32, tag="warm_in")
            nc.vector.memset(warm_in[:], 0.0)
            warm_out = cp_.tile([1, 8], f32, tag="warm_out")
            nc.scalar.activation(warm_out[:], warm_in[:], EXP, scale=1.0)

            # persistent attention outputs O^T per (b, local head): [d, t]
            oT = [[op_.tile([P, T], bf16, tag=f"oT{b}{hh}", name=f"oT{b}{hh}")
                   for hh in range(HPC)] for b in range(B)]

            def outproj_chain(b, hh, cpi):
                csl = slice(cpi * TC_, (cpi + 1) * TC_)
                psy = psc_pool[0].tile([P, TC_], f32, tag="y")
                # stationary: oT columns {t : t%16 == j}, strided view
                ovw = oT[b][hh].rearrange("p (u j) -> p j u", j=KO)
                for j in range(KO):
                    nc.tensor.matmul(psy[:], ovw[:, j, :],
                                     wout_sb[:, j, csl],
                                     start=(j == 0), stop=(j == KO - 1))
                ysb = yp.tile([P, TC_], f32, tag="ysb")
                nc.scalar.copy(ysb[:], psy[:])
                nc.sync.dma_start(
                    y.ap()[b, hh * D:(hh + 1) * D, csl], ysb[:])

            psc_pool = [None]

            for b in range(B):
                qT = [qp.tile([P, T], bf16, tag=f"qT{hh}", name=f"qT{b}{hh}")
                      for hh in range(HPC)]
                kT = [qp.tile([P, T], bf16, tag=f"kT{hh}", name=f"kT{b}{hh}")
                      for hh in range(HPC)]
                vt = [qp.tile([P, NSC, D], bf16, tag=f"v{hh}", name=f"v{b}{hh}")
                      for hh in range(HPC)]

                # ---------------- QKV projection + RoPE ----------------
                with tc.tile_pool(name=f"xt{b}", bufs=2) as xp, \
                     tc.tile_pool(name=f"psA{b}", bufs=3, space="PSUM") as psa, \
                     tc.tile_pool(name=f"psV{b}", bufs=2, space="PSUM") as psv_p, \
                     tc.tile_pool(name=f"rope{b}", bufs=3) as rp:

                    def qkmm(xt, w_sb, hh, nm):
                        hsl = slice(hh * D, (hh + 1) * D)
                        ps = psa.tile([P, TQ], f32, tag="acc", name=nm)
                        for ko in range(KO):
                            nc.tensor.matmul(ps[:], w_sb[:, ko, hsl],
                                             xt[:, ko, :],
                                             start=(ko == 0),
                                             stop=(ko == KO - 1))
                        return ps

                    def rope(ps, dst, sl):
                        # tcos = ps * [cos;cos]; tsw pre-swaps halves:
                        # tsw[0:64]=q2*sin, tsw[64:128]=q1*sin so the add/sub
                        # reads align on base partitions.  All elementwise
                        # work on the VE (bf16 operands get 2x mode).
                        cs = cs_sb[:, sl]
                        sn = sn_sb[:, sl]
                        tcos = rp.tile([P, TQ], bf16, tag="tcos")
                        tsw = rp.tile([P, TQ], bf16, tag="tsw")
                        nc.vector.tensor_mul(tcos[:], ps[:], cs)
                        nc.vector.tensor_mul(tsw[0:HALF, :], ps[HALF:P, :], sn)
                        nc.vector.tensor_mul(tsw[HALF:P, :], ps[0:HALF, :], sn)
                        nc.vector.tensor_sub(dst[0:HALF, sl],
                                             tcos[0:HALF, :], tsw[0:HALF, :])
                        nc.vector.tensor_add(dst[HALF:P, sl],
                                             tcos[HALF:P, :], tsw[HALF:P, :])

                    def vchain(xt, ti):
                        for sub in range(TQ // P):
                            psv = psv_p.tile([P, HPC * D], f32, tag="acc")
                            for ko in range(KO):
                                nc.tensor.matmul(
                                    psv[:], xt[:, ko, sub * P:(sub + 1) * P],
                                    wv_sb[:, ko, :],
                                    start=(ko == 0), stop=(ko == KO - 1))
                            tci = ti * (TQ // P) + sub
                            for hh in range(HPC):
                                # Act engine is idle during QKV; it does the
                                # psum->sbuf v copies.
                                nc.scalar.copy(
                                    vt[hh][:, tci, :],
                                    psv[:, hh * D:(hh + 1) * D])

                    xts = {}
                    for ti in range(NT):
                        xts[ti] = xp.tile([P, KO, TQ], bf16, tag="xt",
                                          name=f"xt{b}_{ti}")

                    if b == 0:
                        # Startup is a DMA-bandwidth wall: ~7MB must land in
                        # the first ~30us.  Chunk the first two x tiles so
                        # chains pace behind arriving data, order DMAs by
                        # first-use time, and defer ti0's v-chains until
                        # after ti1's q/k so wv is needed later.
                        nc.sync.dma_start(xts[0][:, 0:4, :],
                                          xTt.ap()[b, 0, :, 0:4, :])
                        for g in range(1, 4):
                            nc.sync.dma_start(
                                xts[0][:, 4 * g:4 * g + 4, :],
                                xTt.ap()[b, 0, :, 4 * g:4 * g + 4, :])
                        ps = qkmm(xts[0], wq_sb, 0, "acc0_q0")
                        nc.sync.dma_start(wk_sb[:], wk.ap())
                        nc.sync.dma_start(cs_sb[:], cs2.ap())
                        nc.sync.dma_start(sn_sb[:], sn1.ap())
                        rope(ps, qT[0], slice(0, TQ))
                        rope(qkmm(xts[0], wq_sb, 1, "acc0_q1"), qT[1],
                             slice(0, TQ))
                        for g in range(4):
                            nc.sync.dma_start(
                                xts[1][:, 4 * g:4 * g + 4, :],
                                xTt.ap()[b, 1, :, 4 * g:4 * g + 4, :])
                        rope(qkmm(xts[0], wk_sb, 0, "acc0_k0"), kT[0],
                             slice(0, TQ))
                        nc.sync.dma_start(wv_sb[:], wv.ap())
                        nc.sync.dma_start(mask_sb[:], maskM.ap())
                        rope(qkmm(xts[0], wk_sb, 1, "acc0_k1"), kT[1],
                             slice(0, TQ))
                        sl1 = slice(TQ, 2 * TQ)
                        rope(qkmm(xts[1], wq_sb, 0, "acc1_q0"), qT[0], sl1)
                        rope(qkmm(xts[1], wq_sb, 1, "acc1_q1"), qT[1], sl1)
                        rope(qkmm(xts[1], wk_sb, 0, "acc1_k0"), kT[0], sl1)
                        rope(qkmm(xts[1], wk_sb, 1, "acc1_k1"), kT[1], sl1)
                        vchain(xts[1], 1)
                        vchain(xts[0], 0)
                        rest = range(2, NT)
                    else:
                        rest = range(NT)

                    for ti in rest:
                        sl = slice(ti * TQ, (ti + 1) * TQ)
                        xt = xts[ti]
                        nc.sync.dma_start(xt[:], xTt.ap()[b, ti])
                        for hh in range(HPC):
                            rope(qkmm(xt, wq_sb, hh, f"a{ti}q{hh}"),
                                 qT[hh], sl)
                            rope(qkmm(xt, wk_sb, hh, f"a{ti}k{hh}"),
                                 kT[hh], sl)
                        vchain(xt, ti)

                # ------------- attention (S^T layout) + interleaved -----
                # ------------- out-projection of the previous head ------
                # s-chunks are processed in PAIRS sharing one 2-bank PSUM
                # tile and a single exp instruction, so the Act engine
                # (1024 cols + one fixed overhead) runs faster than the
                # PE's 6 matmuls per pair and never paces the pipeline.
                with tc.tile_pool(name=f"psBsc{b}", bufs=2, space="PSUM") as pssc, \
                     tc.tile_pool(name=f"psBo{b}", bufs=2, space="PSUM") as pso, \
                     tc.tile_pool(name=f"psBsum{b}", bufs=2, space="PSUM") as pssum, \
                     tc.tile_pool(name=f"pt{b}", bufs=4) as ptp:
                    nwo = 0   # wout prefetch cursor (b == 0 only)
                    # Deferred-issue queues.  norm_q drains exactly one
                    # t-tile later (ps_o has bufs=2 and the sum bank has
                    # two parity slots, so the normalize MUST be issued
                    # before the slot cycles); op_q drains one per t-tile.
                    norm_q = []
                    op_q = []

                    for hh in range(HPC):
                        for ta in range(NTA):
                            spt = TA // P
                            tsl = slice(ta * TA, (ta + 1) * TA)
                            ps_o = pso.tile([P, TA], f32, tag="o")
                            ps_sum = pssum.tile([1, TA], f32, tag="sum")
                            nblk = (ta + 1) * spt
                            pend = []

                            def flush(last):
                                # same-accumulation-group matmuls must be
                                # adjacent: a LDWEIGHTS that follows an
                                # accumulating matmul whose group is being
                                # suspended stalls ~95ns on hw.  Flush up to
                                # TWO pairs at once, all o-matmuls in one
                                # burst then all sum-matmuls, so only two
                                # group switches happen per flush.
                                take, pend[:] = pend[:2], pend[2:]
                                mms = [(pt_, k, s_, w_)
                                       for pt_, sws in take
                                       for k, (s_, w_) in enumerate(sws)]
                                for i, (pt_, k, s_, w_) in enumerate(mms):
                                    nc.tensor.matmul(ps_o[:, w_],
                                                     vt[hh][:, s_, :],
                                                     pt_[:, k, w_],
                                                     start=(s_ == 0),
                                                     stop=(last and
                                                           i == len(mms) - 1))
                                for i, (pt_, k, s_, w_) in enumerate(mms):
                                    nc.tensor.matmul(
                                        ps_sum[:, w_],
                                        ones_col[:], pt_[:, k, w_],
                                        start=(s_ == 0),
                                        stop=(last and i == len(mms) - 1))

                            for pi in range(nblk // 2):
                                ps_sc = pssc.tile([P, 2, TA], f32, tag="sc")
                                pt = ptp.tile([P, 2, TA], bf16, tag="pt")
                                sws = []
                                for k in range(2):
                                    s = 2 * pi + k
                                    diag = s >= ta * spt
                                    t_lo = (s - ta * spt) * P if diag else 0
                                    w = slice(t_lo, TA)
                                    qsl = slice(ta * TA + t_lo,
                                                (ta + 1) * TA)
                                    nc.tensor.matmul(
                                        ps_sc[:, k, w],
                                        kT[hh][:, s * P:(s + 1) * P],
                                        qT[hh][:, qsl],
                                        start=True, stop=True)
                                    sws.append((s, w))
                                # one exp for both chunks; cols outside a
                                # diag chunk's window hold stale psum ->
                                # garbage pt that no matmul reads
                                nc.scalar.activation(pt[:, :, :],
                                                     ps_sc[:, :, :],
                                                     EXP, scale=SCALE)
                                for k, (s, w) in enumerate(sws):
                                    if s >= ta * spt:  # mask the triangle
                                        t_lo = (s - ta * spt) * P
                                        nc.vector.tensor_mul(
                                            pt[:, k, t_lo:t_lo + P],
                                            pt[:, k, t_lo:t_lo + P],
                                            mask_sb[:])
                                pend.append((pt, sws))
                                if len(pend) > 2:
                                    flush(False)
                                if pi == 0 and norm_q:
                                    norm_q.pop(0)()
                            flush(True)

                            # normalization, deferred: recip on VE,
                            # partition-broadcast on gpsimd, flat multiply
                            # on VE -- issued inside the next tile's block
                            # loop so the PE/Act pipeline never waits.
                            def normalize(ps_o=ps_o, ps_sum=ps_sum,
                                          hh=hh, tsl=tsl):
                                recf = sp.tile([1, TA], f32, tag="recf")
                                nc.vector.reciprocal_approx_fast(
                                    recf[:], ps_sum[:])
                                recb = sp.tile([1, TA], bf16, tag="recb")
                                nc.vector.tensor_copy(recb[:], recf[:])
                                bcb = sp.tile([P, TA], bf16, tag="bcb")
                                nc.gpsimd.partition_broadcast(bcb[:],
                                                              recb[:],
                                                              channels=P)
                                nc.vector.tensor_mul(oT[b][hh][:, tsl],
                                                     ps_o[:], bcb[:])
                            norm_q.append(normalize)

                            if b == 0 and hh == 0:
                                # prefetch all of wout during head 0's
                                # attention (the first out-proj chain needs
                                # every j block)
                                for _ in range(4):
                                    nc.sync.dma_start(
                                        wout_sb[:, nwo, :], woutT.ap()[nwo])
                                    nwo += 1

                    for t_ in norm_q:
                        t_()

                # ---------------- output projection (pure PE) ----------
                with tc.tile_pool(name=f"psC{b}", bufs=2, space="PSUM") as psc:
                    psc_pool[0] = psc
                    for hh in range(HPC):
                        for cpi in range(NCP):
                            outproj_chain(b, hh, cpi)

    nc.compile()
    return nc


_NC = None


def _get_nc():
    global _NC
    if _NC is None:
        _NC = _build()
    return _NC


def _host_tables():
    pos = np.arange(T, dtype=np.float32)[:, None]
    div = np.exp(np.arange(0, 2 * HALF, 2, dtype=np.float32)
                 * np.float32(-math.log(ROPE_BASE) / (2 * HALF)))
    ang = pos * div[None, :]
    cosv = np.cos(ang).astype(np.float32)   # [T, HALF]
    sinv = np.sin(ang).astype(np.float32)
    cosT = np.ascontiguousarray(cosv.T)     # [HALF, T]
    sinT = np.ascontiguousarray(sinv.T)
    cs2 = np.ascontiguousarray(
        np.concatenate([cosT, cosT], axis=0)).astype(ml_dtypes.bfloat16)
    sn1 = np.ascontiguousarray(sinT).astype(ml_dtypes.bfloat16)
    # triangle mask M[s, w] = 1 iff s <= w
    ww = np.arange(P)[None, :]
    ss = np.arange(P)[:, None]
    maskM = (ss <= ww).astype(ml_dtypes.bfloat16)
    return cs2, sn1, maskM


def _make_in_maps(x, Wqkv, Wout):
    x = np.asarray(x, dtype=np.float32)
    Wqkv = np.asarray(Wqkv, dtype=np.float32)
    Wout = np.asarray(Wout, dtype=np.float32)
    assert x.shape == (B, T, C) and Wqkv.shape == (C, 3 * C) \
        and Wout.shape == (C, C)

    cs2, sn1, maskM = _host_tables()
    # xTt[b, ti, p, ko, u] = x[b, ti*TQ+u, ko*128+p]
    xTt = np.ascontiguousarray(
        x.reshape(B, NT, TQ, KO, P).transpose(0, 1, 4, 3, 2)
    ).astype(ml_dtypes.bfloat16)
    woutT = np.ascontiguousarray(
        Wout.reshape(KO, P, C)).astype(ml_dtypes.bfloat16)

    in_maps = []
    for core in range(NCORES):
        h0 = core * HPC
        cols = slice(h0 * D, (h0 + HPC) * D)
        ws = []
        for part in range(3):
            w = Wqkv[:, part * C:(part + 1) * C][:, cols]  # [C, HPC*D]
            ws.append(np.ascontiguousarray(
                w.reshape(KO, P, HPC * D).transpose(1, 0, 2)
            ).astype(ml_dtypes.bfloat16))
        in_maps.append({
            "xTt": xTt,
            "wq": ws[0], "wk": ws[1], "wv": ws[2],
            "woutT": woutT,
            "cs2": cs2, "sn1": sn1, "maskM": maskM,
        })
    return in_maps


def _run(x, Wqkv, Wout, trace=False):
    nc = _get_nc()
    in_maps = _make_in_maps(x, Wqkv, Wout)
    res = run_bass_kernel_spmd(nc, in_maps, core_ids=list(range(NCORES)),
                               trace=trace)
    out = np.empty((B, T, C), dtype=np.float32)
    for core in range(NCORES):
        out[:, core * HPC * D:(core + 1) * HPC * D, :] = \
            res.results[core]["y"]
    return out, res


def kernel(x, Wqkv, Wout):
    out, _ = _run(x, Wqkv, Wout)
    return out
